# revision 13
# baseline (speedup 1.0000x reference)
"""CommNet Trainium2 kernel (8 NeuronCores, data-parallel over batch).

Reference computation (A=32 agents, B=16384 batch, D=64, DA=8, S=3):
    h = tanh(xs @ W_enc^T + b_enc)
    for s in 0..2:
        tot = sum_a h[a]
        others = (tot - h) / (A-1)
        h = tanh(h @ W_h[s]^T + others @ W_c[s]^T)
    out = h @ W_pol^T + b_pol

Device algebra: fold others into
    h @ (W_h - W_c/(A-1))^T + tot @ (W_c/(A-1))^T

On-device layout: D on partitions, tokens on the free axis, two batch
half-chunks stacked on partitions (rows 0-63 chunk A dims, 64-127 chunk B)
so every engine op runs 128 partitions wide. Column order = batch*32+agent
so the agent-sum is a segmented (contiguous) tree reduction on the DVE.
All matmuls bf16 (fp32 PSUM accumulate); tanh on ScalarE; agent tree-sum
on VectorE (bf16 2x mode); policy bias-add on VectorE from PSUM.
"""

import sys
from contextlib import ExitStack

import numpy as np
import ml_dtypes

if "/opt/trn_rl_repo" not in sys.path:
    sys.path.insert(0, "/opt/trn_rl_repo")

BF16 = ml_dtypes.bfloat16

A = 32
B = 16384
D = 64
DA = 8
S = 3
NCORES = 8

BS = B // NCORES          # batches per core
CH = BS // 2              # batches per stacked chunk
COLS = CH * A             # free-axis columns per core
F = 8192                  # columns per streamed tile
GROUP = 2048              # columns per PSUM tile (4 banks)
MMN = 512                 # columns per matmul (1 PSUM bank)

_compiled = {}


def _build(cols, f, group):
    """Build + compile the single-core Bass program (runs SPMD on 8 cores)."""
    import concourse.bass as bass  # noqa: F401
    import concourse.tile as tile
    from concourse import bacc, mybir

    dt = mybir.dt
    Tanh = mybir.ActivationFunctionType.Tanh

    nc = bacc.Bacc("TRN2", target_bir_lowering=False, debug=False)

    xs_ap = nc.dram_tensor("xs", [128, cols], dt.bfloat16, kind="ExternalInput").ap()
    wts_ap = nc.dram_tensor("wts", [128, 928], dt.bfloat16, kind="ExternalInput").ap()
    benc_ap = nc.dram_tensor("benc", [128, 1], dt.float32, kind="ExternalInput").ap()
    bpol_ap = nc.dram_tensor("bpol", [128, 1], dt.float32, kind="ExternalInput").ap()
    out_ap = nc.dram_tensor(
        "out", [128, cols * MMN // group], dt.float32, kind="ExternalOutput"
    ).ap()

    nt = cols // f

    with ExitStack() as ctx:
        tc = ctx.enter_context(tile.TileContext(nc))
        const = ctx.enter_context(tc.tile_pool(name="const", bufs=1))
        xs_pool = ctx.enter_context(tc.tile_pool(name="xsp", bufs=3))
        h_pool = ctx.enter_context(tc.tile_pool(name="hp", bufs=4))
        tree_pool = ctx.enter_context(tc.tile_pool(name="treep", bufs=3))
        tot_pool = ctx.enter_context(
            tc.tile_pool(name="totp", bufs=2 * (f // group) + 1)
        )
        out_pool = ctx.enter_context(tc.tile_pool(name="outp", bufs=2))
        psum = ctx.enter_context(tc.tile_pool(name="psum", bufs=2, space="PSUM"))

        wts = const.tile([128, 928], dt.bfloat16)
        nc.sync.dma_start(wts[:], wts_ap)
        benc = const.tile([128, 1], dt.float32)
        nc.sync.dma_start(benc[:], benc_ap)
        bpol = const.tile([128, 1], dt.float32)
        nc.sync.dma_start(bpol[:], bpol_ap)

        BD_enc = wts[:, 0:128]
        BD_h = [wts[:, 128 * (1 + s):128 * (2 + s)] for s in range(S)]
        BD_c = [wts[:, 128 * (4 + s):128 * (5 + s)] for s in range(S)]
        BD_pol = wts[:, 896:928]

        ng = f // group  # psum groups per tile
        nbg = group // A  # batches (per chunk) in one group

        def agent_tree(nc, h, g):
            """Sum the 32 agents: within a group columns are agent-major
            (col = a*nbg + b), so every tree stage is a flat contiguous
            halves-add -> DVE 2x packed mode."""
            base = g * group
            t16 = tree_pool.tile([128, group // 2], dt.bfloat16, tag="t16")
            nc.vector.tensor_add(
                t16[:], h[:, base:base + group // 2],
                h[:, base + group // 2:base + group],
            )
            t8 = tree_pool.tile([128, group // 4], dt.bfloat16, tag="t8")
            nc.vector.tensor_add(
                t8[:], t16[:, :group // 4], t16[:, group // 4:]
            )
            t4 = tree_pool.tile([128, group // 8], dt.bfloat16, tag="t4")
            nc.vector.tensor_add(t4[:], t8[:, :group // 8], t8[:, group // 8:])
            t2 = tree_pool.tile([128, group // 16], dt.bfloat16, tag="t2")
            nc.vector.tensor_add(t2[:], t4[:, :group // 16], t4[:, group // 16:])
            tot = tot_pool.tile([128, nbg], dt.bfloat16, tag="tot")
            nc.vector.tensor_add(tot[:], t2[:, :nbg], t2[:, nbg:])
            return tot

        for t in range(nt):
            xs_t = xs_pool.tile([128, f], dt.bfloat16, tag="xs")
            for g in range(2 * ng):
                half = group // 2
                nc.sync.dma_start(
                    xs_t[:, g * half:(g + 1) * half],
                    xs_ap[:, t * f + g * half:t * f + (g + 1) * half],
                )

            # encoder: h0 = tanh(BD_enc.T @ xs + b_enc); tree for step 0
            # emitted right after each group's tanh so the DVE work runs
            # while the PE streams the next group.
            h = h_pool.tile([128, f], dt.bfloat16, tag="h")
            tots = []
            for g in range(ng):
                ps = psum.tile([128, group], dt.float32, tag="mm")
                for k in range(group // MMN):
                    c0 = g * group + k * MMN
                    nc.tensor.matmul(
                        ps[:, k * MMN:(k + 1) * MMN],
                        BD_enc,
                        xs_t[:, c0:c0 + MMN],
                        start=True,
                        stop=True,
                    )
                nc.scalar.activation(
                    h[:, g * group:(g + 1) * group], ps[:], Tanh, bias=benc[:]
                )
                tots.append(agent_tree(nc, h, g))

            for s in range(S):
                h_new = h_pool.tile([128, f], dt.bfloat16, tag="h")
                new_tots = []
                for g in range(ng):
                    tot = tots[g]
                    ps = psum.tile([128, group], dt.float32, tag="mm")
                    for k in range(group // MMN):
                        c0 = g * group + k * MMN
                        nc.tensor.matmul(
                            ps[:, k * MMN:(k + 1) * MMN],
                            BD_h[s],
                            h[:, c0:c0 + MMN],
                            start=True,
                            stop=False,
                        )
                    # broadcast tot over the MMN//nbg agents in each chunk
                    for k in range(group // MMN):
                        rhs = (
                            tot[:]
                            .unsqueeze(1)
                            .broadcast_to([128, MMN // nbg, nbg])
                        )
                        nc.tensor.matmul(
                            ps[:, k * MMN:(k + 1) * MMN],
                            BD_c[s],
                            rhs,
                            start=False,
                            stop=True,
                        )
                    nc.scalar.activation(
                        h_new[:, g * group:(g + 1) * group], ps[:], Tanh
                    )
                    if s < S - 1:
                        new_tots.append(agent_tree(nc, h_new, g))
                h = h_new
                tots = new_tots

            # policy head: 4 col-tiled matmuls per group into one
            # [128, 512] psum tile (partition bands 32j..32j+16), then a
            # single full-width bias-add/copy on the DVE.
            for g in range(ng):
                psp = psum.tile([128, MMN], dt.float32, tag="mm")
                for j in range(group // MMN):
                    c0 = g * group + j * MMN
                    nc.tensor.matmul(
                        psp[32 * j:32 * j + 32, :],
                        BD_pol,
                        h[:, c0:c0 + MMN],
                        start=True,
                        stop=True,
                        tile_position=(0, 32 * j),
                    )
                ot = out_pool.tile([128, MMN], dt.float32, tag="ot")
                nc.vector.tensor_scalar_add(ot[:], psp[:], bpol[:])
                gidx = t * ng + g
                nc.sync.dma_start(
                    out_ap[:, gidx * MMN:(gidx + 1) * MMN], ot[:]
                )

    nc.compile()
    return nc


def _get_nc(cols=COLS, f=F, group=GROUP):
    key = (cols, f, group)
    if key not in _compiled:
        _compiled[key] = _build(cols, f, group)
    return _compiled[key]


def _bd(m):
    """Block-diagonal 2x stack of a [k, n] matrix -> [2k, 2n]."""
    k, n = m.shape
    out = np.zeros((2 * k, 2 * n), m.dtype)
    out[:k, :n] = m
    out[k:, n:] = m
    return out


def _host_prep(xs, W_enc, b_enc, W_h, W_c, W_pol, b_pol, bs=BS, group=GROUP,
               ncores=NCORES):
    """Build per-core input maps (layout transform + weight folding).

    Column order per core: two batch half-chunks stacked on partitions;
    within each `group`-column block, columns are agent-major
    (col = a*nbg + b) so the agent tree-sum is contiguous.
    """
    norm = A - 1 if A > 1 else 1
    ch = bs // 2
    nbg = group // A
    wenc_t = W_enc.T.astype(np.float32)
    whp = [(W_h[s] - W_c[s] / norm).T.astype(np.float32) for s in range(S)]
    wcp = [(W_c[s].T / norm).astype(np.float32) for s in range(S)]
    wpol_t = W_pol.T.astype(np.float32)

    wts = np.zeros((128, 928), np.float32)
    wts[:, 0:128] = _bd(wenc_t)
    for s in range(S):
        wts[:, 128 * (1 + s):128 * (2 + s)] = _bd(whp[s])
        wts[:, 128 * (4 + s):128 * (5 + s)] = _bd(wcp[s])
    wts[:, 896:912] = _bd(wpol_t)  # cols 912:928 stay zero (pad to M=32)
    wts = wts.astype(BF16)

    benc = np.concatenate([b_enc, b_enc]).reshape(128, 1).astype(np.float32)
    # policy bias bands: partitions 32j+dd, dd<8 chunk A, 8<=dd<16 chunk B
    bpol = np.zeros((128, 1), np.float32)
    for j in range(group // MMN):
        bpol[32 * j:32 * j + DA, 0] = b_pol
        bpol[32 * j + DA:32 * j + 2 * DA, 0] = b_pol

    def chunk_layout(xc):  # [D, ch, A] -> [D, ch*A] agent-major per group
        ngrp = ch // nbg
        return (
            xc.reshape(D, ngrp, nbg, A)
            .transpose(0, 1, 3, 2)
            .reshape(D, ch * A)
        )

    in_maps = []
    for c in range(ncores):
        xc = xs[:, c * bs:(c + 1) * bs, :]            # [A, bs, D]
        xt = np.ascontiguousarray(xc.transpose(2, 1, 0))  # [D, bs, A]
        cA = chunk_layout(xt[:, :ch, :])
        cB = chunk_layout(xt[:, ch:, :])
        xs_t = np.concatenate([cA, cB], axis=0).astype(BF16)  # [128, cols]
        in_maps.append({"xs": xs_t, "wts": wts, "benc": benc, "bpol": bpol})
    return in_maps


def _host_gather(results, bs=BS, group=GROUP, ncores=NCORES):
    """Per-core [128, ngrp*MMN] banded policy outputs -> [A, B, DA] f32."""
    ch = bs // 2
    nbg = group // A
    ngrp = ch // nbg
    jn = group // MMN
    pj = A // jn  # agents per policy col-tile band
    outs = []
    for c in range(ncores):
        r = results[c]["out"]                          # [128, ngrp*MMN]
        arr = r.reshape(jn, 128 // jn, ngrp, pj, nbg)[:, :2 * DA]
        arr = arr.reshape(jn, 2, DA, ngrp, pj, nbg)    # j, chunk, d, g, a', b
        oc = arr.transpose(0, 4, 1, 3, 5, 2).reshape(A, bs, DA)
        outs.append(oc)
    return np.concatenate(outs, axis=1).astype(np.float32)


def kernel(xs, W_enc, b_enc, W_h, W_c, W_pol, b_pol, _trace=False):
    from concourse.bass_utils import run_bass_kernel_spmd

    xs = np.asarray(xs, np.float32)
    in_maps = _host_prep(
        xs,
        np.asarray(W_enc, np.float32),
        np.asarray(b_enc, np.float32),
        np.asarray(W_h, np.float32),
        np.asarray(W_c, np.float32),
        np.asarray(W_pol, np.float32),
        np.asarray(b_pol, np.float32),
    )
    nc = _get_nc()
    res = run_bass_kernel_spmd(
        nc, in_maps, core_ids=list(range(NCORES)), trace=_trace
    )
    out = _host_gather(res.results)
    if _trace:
        return out, res
    return out
